# revision 22
# baseline (speedup 1.0000x reference)
"""Trainium2 Bass kernel for a GNN message-passing layer — fp8 DoubleRow variant.

Same math/decomposition as the f16 kernel (one-hot scatter matmuls into
per-destination-window PSUM sums, then a per-node GEMM with combined weights,
LayerNorm tail), with two changes:

1. The per-message stream [Hh | E] is quantized to fp8(e4m3) and the scatter
   matmuls run in DoubleRow mode (256-message contraction per tile, 2 fp8
   MACs/cell/cycle) — half the PE time and half the stream DMA.
2. Low-degree nodes (total degree <= C_LOW) are precision-sensitive (their
   aggregate is a mean of few messages, so fp8 quantization error doesn't
   average out); they are packed into a few dedicated windows processed on the
   f16 path.

The one-hot scatter matrices are built on the host (pure sharding metadata)
and streamed as fp8/f16, freeing the vector engine for the Hh*E products.
"""

import os
import numpy as np
import ml_dtypes

import concourse.bass as bass
import concourse.bacc as bacc
import concourse.mybir as mybir
import concourse.tile as tile
from concourse.bass_utils import run_bass_kernel_spmd

N_NODES = 50000
N_EDGES = 250000
D = 256
LEAKY = 0.01
LN_EPS = 1e-5

N_CORES = 8
WPC8 = 46        # fp8 windows per core
WPC16 = 4        # f16 (low-degree) windows per core
WPC = WPC8 + WPC16
NWIN = N_CORES * WPC
N8T = 3          # fp8 double-tiles (256 msgs) per window-direction
T16 = 3          # f16 tiles (128 msgs) per window-direction
C_LOW = 5        # nodes with total degree <= C_LOW go to f16 windows

PROFILE = bool(int(os.environ.get("KERNEL_TRACE", "0")))
LAST = {}

F8NP = ml_dtypes.float8_e4m3
F32 = mybir.dt.float32
F16 = mybir.dt.float16
F8 = mybir.dt.float8e4
DR = mybir.MatmulPerfMode.DoubleRow


# ----------------------------------------------------------------- host side

def _positions_in_group(group_ids, n_groups):
    order = np.argsort(group_ids, kind="stable")
    counts = np.bincount(group_ids, minlength=n_groups)
    starts = np.zeros(n_groups + 1, dtype=np.int64)
    np.cumsum(counts, out=starts[1:])
    pos = np.arange(len(group_ids), dtype=np.int64) - starts[group_ids[order]]
    return order, pos, counts


def _assign_cores(cnt, low):
    """Balance nodes across cores by message load, with per-class slot caps."""
    hi_cap = WPC8 * 128
    lo_cap = WPC16 * 128
    load = np.zeros(N_CORES)
    hi_n = np.zeros(N_CORES, dtype=np.int64)
    lo_n = np.zeros(N_CORES, dtype=np.int64)
    core_of = np.empty(N_NODES, dtype=np.int64)
    order = np.argsort(-cnt, kind="stable")
    for v in order:
        if low[v]:
            c = int(np.argmin(np.where(lo_n < lo_cap, load, np.inf)))
            lo_n[c] += 1
        else:
            c = int(np.argmin(np.where(hi_n < hi_cap, load, np.inf)))
            hi_n[c] += 1
        core_of[v] = c
        load[c] += cnt[v]
    return core_of


def _pack_core(nodes, cnt_f, cnt_b, nwin, cap):
    """Pack the given nodes into nwin windows (<=128 nodes each) keeping
    per-direction message counts <= cap. Greedy min-max."""
    F = np.zeros(nwin)
    B = np.zeros(nwin)
    NN = np.zeros(nwin, dtype=np.int64)
    slot = np.empty(len(nodes), dtype=np.int64)
    loc = np.empty(len(nodes), dtype=np.int64)
    order = np.argsort(-(cnt_f[nodes] + cnt_b[nodes]), kind="stable")
    for i in order:
        v = nodes[i]
        cf = cnt_f[v]
        cb = cnt_b[v]
        score = np.maximum(F + cf, B + cb)
        bad = (NN >= 128) | (F + cf > cap) | (B + cb > cap)
        score = np.where(bad, np.inf, score)
        w = int(np.argmin(score))
        if not np.isfinite(score[w]):
            return None
        slot[i] = w
        loc[i] = NN[w]
        F[w] += cf
        B[w] += cb
        NN[w] += 1
    return slot, loc


def _pack_host(H, E, ht):
    heads = ht[:, 0].astype(np.int64)
    tails = ht[:, 1].astype(np.int64)
    cnt_f = np.bincount(tails, minlength=N_NODES)
    cnt_b = np.bincount(heads, minlength=N_NODES)
    cnt = cnt_f + cnt_b

    c_low = C_LOW
    low = cnt <= c_low
    while low.sum() > N_CORES * WPC16 * 128 and c_low > 0:
        c_low -= 1
        low = cnt <= c_low

    core_of = _assign_cores(cnt, low)

    win_of = np.empty(N_NODES, dtype=np.int64)   # global window id
    loc_of = np.empty(N_NODES, dtype=np.int64)
    for c in range(N_CORES):
        hi_nodes = np.where((core_of == c) & ~low)[0]
        r = _pack_core(hi_nodes, cnt_f, cnt_b, WPC8, 256 * N8T)
        if r is None:
            return None
        slot, loc = r
        win_of[hi_nodes] = c * WPC + slot
        loc_of[hi_nodes] = loc
        lo_nodes = np.where((core_of == c) & low)[0]
        r = _pack_core(lo_nodes, cnt_f, cnt_b, WPC16, 128 * T16)
        if r is None:
            return None
        slot, loc = r
        win_of[lo_nodes] = c * WPC + WPC8 + slot
        loc_of[lo_nodes] = loc

    H8 = H.astype(F8NP)
    E8 = E.astype(F8NP)
    H16 = H.astype(np.float16)
    E16 = E.astype(np.float16)

    n8rows = N_CORES * WPC8 * 2 * N8T * 2    # (core,win8,dir,t,ko) 128-slot rows
    s8 = np.zeros((n8rows * 128, 512), dtype=F8NP)
    i8 = np.zeros((n8rows * 128, 128), dtype=F8NP)
    n16rows = N_CORES * WPC16 * 2 * T16
    s16 = np.zeros((n16rows * 128, 512), dtype=np.float16)
    i16 = np.zeros((n16rows * 128, 128), dtype=np.float16)

    for d, (src, dst) in enumerate(((heads, tails), (tails, heads))):
        w = win_of[dst]
        order, pos, _counts = _positions_in_group(w, NWIN)
        e_s = order
        w_s = w[order]
        core = w_s // WPC
        slot = w_s % WPC
        is8 = slot < WPC8

        m = is8
        w8g = core[m] * WPC8 + slot[m]
        p = pos[m]
        t = p // 256
        ko = (p % 256) // 128
        ki = p % 128
        row = ((w8g * 2 + d) * N8T + t) * 2 + ko
        flat = row * 128 + ki
        s8[flat, 0:256] = H8[src[e_s[m]]]
        s8[flat, 256:512] = E8[e_s[m]]
        i8[flat, loc_of[dst[e_s[m]]]] = 1.0

        m = ~is8
        w16g = core[m] * WPC16 + (slot[m] - WPC8)
        p = pos[m]
        t = p // 128
        ki = p % 128
        row = (w16g * 2 + d) * T16 + t
        flat = row * 128 + ki
        s16[flat, 0:256] = H16[src[e_s[m]]]
        s16[flat, 256:512] = E16[e_s[m]]
        i16[flat, loc_of[dst[e_s[m]]]] = 1.0

    # device layouts (partition dim = 128 slot-lanes); ind is appended to the
    # stream row so each window-direction is a single contiguous 2D DMA
    s8 = s8.reshape(N_CORES, WPC8 * 2, N8T * 2, 128, 512)
    s8 = s8.transpose(0, 1, 3, 2, 4).reshape(N_CORES, WPC8 * 2, 128,
                                             N8T * 2 * 512)
    i8 = i8.reshape(N_CORES, WPC8 * 2, N8T * 2, 128, 128)
    i8 = i8.transpose(0, 1, 3, 2, 4).reshape(N_CORES, WPC8 * 2, 128,
                                             N8T * 2 * 128)
    s8 = np.ascontiguousarray(np.concatenate([s8, i8], axis=3))
    s16 = s16.reshape(N_CORES, WPC16 * 2, T16, 128, 512)
    s16 = s16.transpose(0, 1, 3, 2, 4).reshape(N_CORES, WPC16 * 2, 128,
                                               T16 * 512)
    i16 = i16.reshape(N_CORES, WPC16 * 2, T16, 128, 128)
    i16 = i16.transpose(0, 1, 3, 2, 4).reshape(N_CORES, WPC16 * 2, 128,
                                               T16 * 128)
    s16 = np.ascontiguousarray(np.concatenate([s16, i16], axis=3))

    node_ids = np.full((NWIN, 128), -1, dtype=np.int64)
    node_ids[win_of, loc_of] = np.arange(N_NODES, dtype=np.int64)

    recip_all = 1.0 / np.maximum(cnt, 1).astype(np.float32)
    safe_ids = np.maximum(node_ids, 0)
    hres = H[safe_ids].astype(np.float16)
    hres[node_ids < 0] = 0.0
    recip = recip_all[safe_ids]
    recip[node_ids < 0] = 1.0

    hres = hres.reshape(N_CORES, WPC * 128, D)
    recip = recip.reshape(N_CORES, WPC, 128).transpose(0, 2, 1).copy()
    return {
        "s8": s8, "s16": s16,
        "hres": hres, "recip": recip, "node_ids": node_ids,
        "cnt_f": cnt_f, "cnt_b": cnt_b, "cnt": cnt,
    }


def _weights_pack(W_fwd, W_back):
    def cat(W):
        W1, W2, W3, W4 = (W[:, i * D:(i + 1) * D] for i in range(4))
        return np.concatenate([(W1 + W3).T, (W2 + W3).T, W4.T], axis=0)

    # per-direction acc halves [Hh|E|He] -> natural block order for both
    wf = np.ascontiguousarray(cat(W_fwd).reshape(6, 128, D), dtype=np.float16)
    wb = np.ascontiguousarray(cat(W_back).reshape(6, 128, D), dtype=np.float16)
    return wf, wb


# --------------------------------------------------------------- device side

def _build_nc(use_bias, use_gb):
    nc = bacc.Bacc()

    SW8 = N8T * 2 * 512                  # stream cols per fp8 window-dir
    IW8 = N8T * 2 * 128                  # ind cols
    SW16 = T16 * 512
    IW16 = T16 * 128
    s8_d = nc.dram_tensor("s8", [WPC8 * 2, 128, SW8 + IW8], F8,
                          kind="ExternalInput")
    s16_d = nc.dram_tensor("s16", [WPC16 * 2, 128, SW16 + IW16], F16,
                           kind="ExternalInput")
    hres_d = nc.dram_tensor("hres", [WPC * 128, D], F16, kind="ExternalInput")
    recip_d = nc.dram_tensor("recip", [128, WPC], F32, kind="ExternalInput")
    wf_d = nc.dram_tensor("wf", [6, 128, D], F16, kind="ExternalInput")
    wb_d = nc.dram_tensor("wb", [6, 128, D], F16, kind="ExternalInput")
    ident_d = nc.dram_tensor("ident", [128, 128], F16, kind="ExternalInput")
    if use_bias:
        bc_d = nc.dram_tensor("bc", [WPC * 128, D], F32, kind="ExternalInput")
    if use_gb:
        gam_d = nc.dram_tensor("gam", [1, D], F32, kind="ExternalInput")
        bet_d = nc.dram_tensor("bet", [1, D], F32, kind="ExternalInput")
    out_d = nc.dram_tensor("out", [WPC * 128, D], F16, kind="ExternalOutput")

    with tile.TileContext(nc) as tc:
        with (
            tc.tile_pool(name="const", bufs=1) as constp,
            tc.tile_pool(name="st8", bufs=8) as st8p,
            tc.tile_pool(name="he8", bufs=4) as he8p,
            tc.tile_pool(name="st16", bufs=6) as st16p,
            tc.tile_pool(name="he16", bufs=2) as he16p,
            tc.tile_pool(name="aggsb", bufs=4) as aggsbp,
            tc.tile_pool(name="aggT", bufs=4) as aggTp,
            tc.tile_pool(name="tailp", bufs=3) as tailp,
            tc.tile_pool(name="outp", bufs=4) as outp,
            tc.tile_pool(name="hresp", bufs=4) as hresp,
            tc.tile_pool(name="pacc", bufs=2, space="PSUM") as pacc,
            tc.tile_pool(name="ptp", bufs=2, space="PSUM") as ptp,
            tc.tile_pool(name="pnd", bufs=2, space="PSUM") as pnd,
        ):
            ident = constp.tile([128, 128], F16)
            nc.sync.dma_start(out=ident, in_=ident_d[:, :])
            wf_sb = constp.tile([128, 6, D], F16)
            nc.sync.dma_start(out=wf_sb,
                              in_=wf_d[:, :, :].rearrange("c k n -> k c n"))
            wb_sb = constp.tile([128, 6, D], F16)
            nc.sync.dma_start(out=wb_sb,
                              in_=wb_d[:, :, :].rearrange("c k n -> k c n"))
            recip_sb = constp.tile([128, WPC], F32)
            nc.sync.dma_start(out=recip_sb, in_=recip_d[:, :])
            eps_sb = constp.tile([128, 1], F32)
            nc.vector.memset(eps_sb, LN_EPS)
            if use_gb:
                gam_sb = constp.tile([128, D], F32)
                nc.sync.dma_start(
                    out=gam_sb,
                    in_=bass.AP(tensor=gam_d, offset=0, ap=[[0, 128], [1, D]]),
                )
                bet_sb = constp.tile([128, D], F32)
                nc.sync.dma_start(
                    out=bet_sb,
                    in_=bass.AP(tensor=bet_d, offset=0, ap=[[0, 128], [1, D]]),
                )

            def reduce_dir(d, acc, nodeps):
                aggsb = aggsbp.tile([128, 768], F16)
                nc.scalar.copy(out=aggsb, in_=acc)

                aggT = aggTp.tile([128, 6, 128], F16)
                tp = ptp.tile([128, 768], F16)
                for j in range(6):
                    nc.tensor.transpose(
                        tp[:, j * 128:(j + 1) * 128],
                        aggsb[:, j * 128:(j + 1) * 128], ident,
                    )
                nc.scalar.copy(out=aggT, in_=tp)

                wsb = wf_sb if d == 0 else wb_sb
                for blk in range(6):
                    nc.tensor.matmul(
                        nodeps, aggT[:, blk, :], wsb[:, blk, :],
                        start=(d == 0 and blk == 0),
                        stop=(d == 1 and blk == 5),
                    )

            def tail(w, nodeps):
                x = tailp.tile([128, D], F32, tag="x")
                if use_bias:
                    y = tailp.tile([128, D], F32, tag="y")
                    nc.scalar.activation(
                        out=y, in_=nodeps,
                        func=mybir.ActivationFunctionType.Copy,
                        bias=0.0, scale=recip_sb[:, w:w + 1],
                    )
                    bc_sb = tailp.tile([128, D], F32, tag="bc")
                    nc.sync.dma_start(
                        out=bc_sb, in_=bc_d[w * 128:(w + 1) * 128, :])
                    nc.vector.tensor_add(y, y, bc_sb)
                    nc.scalar.activation(
                        out=x, in_=y,
                        func=mybir.ActivationFunctionType.Prelu,
                        bias=0.0, scale=1.0, alpha=LEAKY,
                    )
                else:
                    nc.scalar.activation(
                        out=x, in_=nodeps,
                        func=mybir.ActivationFunctionType.Prelu,
                        bias=0.0, scale=recip_sb[:, w:w + 1], alpha=LEAKY,
                    )

                hres_sb = hres_tiles.pop(w)
                nc.gpsimd.tensor_add(x, x, hres_sb)

                stats = tailp.tile([128, 6], F32, tag="stats")
                nc.vector.bn_stats(out=stats, in_=x)
                mv = tailp.tile([128, 2], F32, tag="mv")
                nc.vector.bn_aggr(out=mv, in_=stats)
                std = tailp.tile([128, 1], F32, tag="std")
                nc.scalar.activation(
                    out=std, in_=mv[:, 1:2],
                    func=mybir.ActivationFunctionType.Sqrt,
                    bias=eps_sb, scale=1.0,
                )
                rstd = tailp.tile([128, 1], F32, tag="rstd")
                nc.vector.reciprocal(out=rstd, in_=std)
                nmr = tailp.tile([128, 1], F32, tag="nmr")
                nc.vector.tensor_scalar(
                    out=nmr, in0=mv[:, 0:1], scalar1=rstd, scalar2=-1.0,
                    op0=mybir.AluOpType.mult, op1=mybir.AluOpType.mult,
                )

                o = outp.tile([128, D], F32 if use_gb else F16)
                nc.vector.tensor_scalar(
                    out=o, in0=x, scalar1=rstd, scalar2=nmr,
                    op0=mybir.AluOpType.mult, op1=mybir.AluOpType.add,
                )
                if use_gb:
                    o2 = outp.tile([128, D], F16, tag="o2")
                    nc.vector.tensor_tensor(
                        out=o, in0=o, in1=gam_sb, op=mybir.AluOpType.mult)
                    nc.vector.tensor_tensor(
                        out=o2, in0=o, in1=bet_sb, op=mybir.AluOpType.add)
                    o = o2
                nc.sync.dma_start(
                    out=out_d[w * 128:(w + 1) * 128, :], in_=o)

            def view3(sl, dims):
                # rebuild a tile slice as a 3D AP [partition, mid, inner]
                return bass.AP(tensor=sl.tensor, offset=sl.offset,
                               ap=[list(sl.ap[0])] + [list(dd) for dd in dims])

            # stream DMAs are emitted two windows ahead so the sync queue's
            # in-order waits (e.g. out-DMA waiting on the LN tail) never gate
            # the prefetch of upcoming windows
            PF8 = 3
            st8_tiles = {}

            hres_tiles = {}

            def prefetch_hres(w):
                h = hresp.tile([128, D], F16, tag="hres")
                nc.sync.dma_start(out=h, in_=hres_d[w * 128:(w + 1) * 128, :])
                hres_tiles[w] = h

            def prefetch8(w):
                for d in range(2):
                    st = st8p.tile([128, SW8 + IW8], F8, tag="st8")
                    nc.sync.dma_start(out=st, in_=s8_d[w * 2 + d, :, :])
                    st8_tiles[(w, d)] = st
                prefetch_hres(w)

            for w in range(min(PF8, WPC8)):
                prefetch8(w)

            for w in range(WPC8):
                if w + PF8 < WPC8:
                    prefetch8(w + PF8)
                nodeps = pnd.tile([128, D], F32)
                for d in range(2):
                    acc = pacc.tile([128, 768], F32)
                    st = st8_tiles.pop((w, d))
                    he = he8p.tile([128, N8T * 2 * 256], F8, tag="he8")
                    nc.vector.tensor_tensor(
                        out=view3(he[:, :], [[256, N8T * 2], [1, 256]]),
                        in0=view3(st[:, 0:SW8], [[512, N8T * 2], [1, 256]]),
                        in1=view3(st[:, 256:SW8], [[512, N8T * 2], [1, 256]]),
                        op=mybir.AluOpType.mult,
                    )
                    for t in range(N8T):
                        lhs = view3(st[:, SW8 + t * 256:SW8 + (t + 1) * 256],
                                    [[128, 2], [1, 128]])
                        rhs = view3(st[:, t * 1024:(t + 1) * 1024],
                                    [[512, 2], [1, 512]])
                        rhe = view3(he[:, t * 512:(t + 1) * 512],
                                    [[256, 2], [1, 256]])
                        nc.tensor.matmul(
                            acc[:, 0:512], lhs, rhs,
                            start=(t == 0), stop=(t == N8T - 1),
                            perf_mode=DR,
                        )
                        nc.tensor.matmul(
                            acc[:, 512:768], lhs, rhe,
                            start=(t == 0), stop=(t == N8T - 1),
                            perf_mode=DR,
                        )
                    reduce_dir(d, acc, nodeps)
                tail(w, nodeps)

            PF16 = 2
            st16_tiles = {}

            def prefetch16(s):
                for d in range(2):
                    st = st16p.tile([128, SW16 + IW16], F16, tag="st16")
                    nc.sync.dma_start(out=st, in_=s16_d[s * 2 + d, :, :])
                    st16_tiles[(s, d)] = st
                prefetch_hres(WPC8 + s)

            for s in range(min(PF16, WPC16)):
                prefetch16(s)

            for s in range(WPC16):
                if s + PF16 < WPC16:
                    prefetch16(s + PF16)
                w = WPC8 + s
                nodeps = pnd.tile([128, D], F32)
                for d in range(2):
                    acc = pacc.tile([128, 768], F32)
                    st = st16_tiles.pop((s, d))
                    he = he16p.tile([128, T16 * 256], F16, tag="he16")
                    nc.vector.tensor_tensor(
                        out=view3(he[:, :], [[256, T16], [1, 256]]),
                        in0=view3(st[:, 0:SW16], [[512, T16], [1, 256]]),
                        in1=view3(st[:, 256:SW16], [[512, T16], [1, 256]]),
                        op=mybir.AluOpType.mult,
                    )
                    for t in range(T16):
                        nc.tensor.matmul(
                            acc[:, 0:512],
                            st[:, SW16 + t * 128:SW16 + (t + 1) * 128],
                            st[:, t * 512:(t + 1) * 512],
                            start=(t == 0), stop=(t == T16 - 1),
                        )
                        nc.tensor.matmul(
                            acc[:, 512:768],
                            st[:, SW16 + t * 128:SW16 + (t + 1) * 128],
                            he[:, t * 256:(t + 1) * 256],
                            start=(t == 0), stop=(t == T16 - 1),
                        )
                    reduce_dir(d, acc, nodeps)
                tail(w, nodeps)

    nc.compile()
    return nc


_NC_CACHE = {}


def kernel(H, E, ht, W_fwd, b_fwd, W_back, b_back, gamma, beta):
    H = np.asarray(H, dtype=np.float32)
    E = np.asarray(E, dtype=np.float32)
    ht = np.asarray(ht)
    W_fwd = np.asarray(W_fwd, dtype=np.float32)
    W_back = np.asarray(W_back, dtype=np.float32)
    b_fwd = np.asarray(b_fwd, dtype=np.float32)
    b_back = np.asarray(b_back, dtype=np.float32)
    gamma = np.asarray(gamma, dtype=np.float32)
    beta = np.asarray(beta, dtype=np.float32)

    pk = _pack_host(H, E, ht)
    assert pk is not None, "window packing failed"

    wf, wb = _weights_pack(W_fwd, W_back)
    use_bias = bool(np.any(b_fwd) or np.any(b_back))
    use_gb = bool(np.any(gamma != 1.0) or np.any(beta != 0.0))

    key = (use_bias, use_gb)
    if key not in _NC_CACHE:
        _NC_CACHE[key] = _build_nc(use_bias, use_gb)
    nc = _NC_CACHE[key]

    ident = np.eye(128, dtype=np.float16)

    in_maps = []
    for c in range(N_CORES):
        m = {
            "s8": pk["s8"][c],
            "s16": pk["s16"][c],
            "hres": pk["hres"][c],
            "recip": pk["recip"][c],
            "wf": wf,
            "wb": wb,
            "ident": ident,
        }
        if use_bias:
            recip_all = 1.0 / np.maximum(pk["cnt"], 1).astype(np.float32)
            bcv = (pk["cnt_f"][:, None] * b_fwd[None, :]
                   + pk["cnt_b"][:, None] * b_back[None, :]) \
                * recip_all[:, None]
            ids = pk["node_ids"].reshape(NWIN, 128)
            safe = np.maximum(ids, 0)
            bc = bcv[safe]
            bc[ids < 0] = 0.0
            m["bc"] = np.ascontiguousarray(
                bc.reshape(N_CORES, WPC * 128, D)[c], dtype=np.float32)
        if use_gb:
            m["gam"] = gamma.reshape(1, D)
            m["bet"] = beta.reshape(1, D)
        in_maps.append(m)

    kwargs = {}
    if PROFILE:
        try:
            import antenv.axon_hooks  # noqa: F401
            kwargs = dict(trace=True, trace_cores=[0])
        except ImportError:
            pass
    res = run_bass_kernel_spmd(nc, in_maps, core_ids=list(range(N_CORES)),
                               **kwargs)
    LAST["exec_time_ns"] = res.exec_time_ns
    LAST["results"] = res

    out = np.empty((N_NODES, D), dtype=np.float32)
    ids = pk["node_ids"]  # [NWIN, 128]
    for c in range(N_CORES):
        rows = res.results[c]["out"]  # [WPC*128, D] f16
        wids = ids[c * WPC:(c + 1) * WPC].reshape(-1)
        valid = wids >= 0
        out[wids[valid]] = rows[valid].astype(np.float32)
    return out
